# revision 5
# baseline (speedup 1.0000x reference)
"""MoE feed-forward (dense-routing reference) on 8 trn2 NeuronCores.

Strategy: expert-parallel, one expert per core, with top-2 sparsity
exploited — only tokens routed to an expert are sent to its core.

Host side (numpy): gating (fp64 logits -> softmax -> top-2; fp64 ordering
agrees with every fp32 backend unless the 2nd/3rd gap is ~1e-6, verified
safe for this input distribution), gather of each expert's tokens into a
transposed [H, C] batch, and scatter-add of the per-expert outputs.

Device side (Bass/Tile, per core):
  hiddenT[f, tok] = relu(w1[h, f].T @ xT[h, tok] + b1[f])     (bf16 matmul)
  y[tok, h]      = (hiddenT[f, tok].T @ w2[f, h]) * gate[tok] (bf16 matmul)
Weights stay resident in SBUF as bf16; tokens stream in chunks of 512.
"""

import numpy as np

import concourse.bass as bass
import concourse.tile as tile
from concourse import bacc, mybir
from concourse.bass_utils import run_bass_kernel_spmd

P = 128
H = 1024
F = 4096
E = 8
TOPK = 2
HT = H // P   # 8 contraction tiles for MM1
FT = F // P   # 32 contraction tiles for MM2
TOK = 512     # token chunk (moving-dim N for MM1)
NH = 512      # output column tile for MM2

PROFILE = False          # test harness sets True to get a trace
LAST_RESULT = None       # BassKernelResults of last run (when PROFILE)

_nc_cache = {}


def _build(C):
    """Build + compile the per-core SPMD program for token capacity C."""
    assert C % P == 0
    f32 = mybir.dt.float32
    bf16 = mybir.dt.bfloat16

    nc = bacc.Bacc("TRN2", target_bir_lowering=False, debug=False)
    xT_d = nc.dram_tensor("xT", [H, C], f32, kind="ExternalInput")
    w1_d = nc.dram_tensor("w1", [H, F], f32, kind="ExternalInput")
    b1_d = nc.dram_tensor("b1", [P, FT], f32, kind="ExternalInput")
    w2_d = nc.dram_tensor("w2", [F, H], f32, kind="ExternalInput")
    sc_d = nc.dram_tensor("sc", [P, C // P], f32, kind="ExternalInput")
    y_d = nc.dram_tensor("y", [C, H], f32, kind="ExternalOutput")

    chunks = []
    c0 = 0
    while c0 < C:
        t = min(TOK, C - c0)
        chunks.append((c0, t))
        c0 += t

    with tile.TileContext(nc) as tc:
        with (
            tc.tile_pool(name="wres", bufs=1) as wres,
            tc.tile_pool(name="consts", bufs=1) as consts,
            tc.tile_pool(name="stage", bufs=3) as stage,
            tc.tile_pool(name="xstage", bufs=3) as xstage,
            tc.tile_pool(name="xpool", bufs=1) as xpool,
            tc.tile_pool(name="hidp", bufs=1) as hidp,
            tc.tile_pool(name="outp", bufs=3) as outp,
            tc.tile_pool(name="psum1", bufs=3, space="PSUM") as psum1,
            tc.tile_pool(name="psum2", bufs=3, space="PSUM") as psum2,
        ):
            w1_sb = wres.tile([P, HT, F], bf16)      # 64KB/partition
            w2_sb = wres.tile([P, FT, H], bf16)      # 64KB/partition
            b1_sb = consts.tile([P, FT], f32)
            sc_sb = consts.tile([P, C // P], f32)

            nc.gpsimd.dma_start(out=b1_sb, in_=b1_d[:, :])
            nc.gpsimd.dma_start(out=sc_sb, in_=sc_d[:, :])

            # Resident weights: DMA fp32 pieces, cast to bf16.
            # w1 pieces emitted f-major so MM1 of chunk 0 can start early.
            for q in range(4):
                for h in range(HT):
                    st = stage.tile([P, 1024], f32, tag="stage")
                    nc.sync.dma_start(
                        out=st, in_=w1_d[h * P:(h + 1) * P, q * 1024:(q + 1) * 1024]
                    )
                    nc.vector.tensor_copy(
                        out=w1_sb[:, h, q * 1024:(q + 1) * 1024], in_=st
                    )
            for i in range(FT):
                st = stage.tile([P, 1024], f32, tag="stage")
                nc.gpsimd.dma_start(out=st, in_=w2_d[i * P:(i + 1) * P, :])
                nc.vector.tensor_copy(out=w2_sb[:, i, :], in_=st)

            for (c0, t) in chunks:
                # Load + cast this chunk of xT.
                xb = xpool.tile([P, HT, TOK], bf16, tag="xb")
                hid_sb = hidp.tile([P, FT, TOK], bf16, tag="hid")
                for h in range(HT):
                    st = xstage.tile([P, TOK], f32, tag="xstage")
                    nc.scalar.dma_start(
                        out=st[:, :t], in_=xT_d[h * P:(h + 1) * P, c0:c0 + t]
                    )
                    nc.vector.tensor_copy(out=xb[:, h, :t], in_=st[:, :t])

                # MM1: hiddenT[f, tok] = relu(w1.T @ xT + b1)
                for i in range(FT):
                    ps = psum1.tile([P, TOK], f32, tag="ps1")
                    for h in range(HT):
                        nc.tensor.matmul(
                            ps[:, :t],
                            w1_sb[:, h, i * P:(i + 1) * P],
                            xb[:, h, :t],
                            start=(h == 0),
                            stop=(h == HT - 1),
                        )
                    nc.scalar.activation(
                        hid_sb[:, i, :t],
                        ps[:, :t],
                        mybir.ActivationFunctionType.Relu,
                        bias=b1_sb[:, i:i + 1],
                    )

                # MM2: y[tok, h] = (hiddenT.T @ w2) * gate
                for m in range(t // P):
                    mg = c0 // P + m
                    for nh in range(H // NH):
                        ps = psum2.tile([P, NH], f32, tag="ps2")
                        for i in range(FT):
                            nc.tensor.matmul(
                                ps,
                                hid_sb[:, i, m * P:(m + 1) * P],
                                w2_sb[:, i, nh * NH:(nh + 1) * NH],
                                start=(i == 0),
                                stop=(i == FT - 1),
                            )
                        ot = outp.tile([P, NH], f32, tag="out")
                        nc.vector.tensor_scalar_mul(ot, ps, sc_sb[:, mg:mg + 1])
                        nc.sync.dma_start(
                            out=y_d[c0 + m * P:c0 + (m + 1) * P,
                                    nh * NH:(nh + 1) * NH],
                            in_=ot,
                        )

    nc.compile()
    return nc


def _get_nc(C):
    if C not in _nc_cache:
        _nc_cache[C] = _build(C)
    return _nc_cache[C]


def kernel(x, gate_w, w1, b1, w2, b2):
    global LAST_RESULT
    S, B, _ = x.shape
    T = S * B
    xf = np.ascontiguousarray(x.reshape(T, H), dtype=np.float32)

    # --- host gating (fp64 for backend-stable ordering) ---
    l64 = xf.astype(np.float64) @ gate_w.T.astype(np.float64)
    l64 -= l64.max(axis=-1, keepdims=True)
    p64 = np.exp(l64)
    p64 /= p64.sum(axis=-1, keepdims=True)
    top2 = np.argsort(-l64, axis=-1)[:, :TOPK]
    probs = p64.astype(np.float32)

    xfT = np.ascontiguousarray(xf.T)  # [H, T]
    onehot = np.zeros((T, E), dtype=bool)
    onehot[np.arange(T)[:, None], top2] = True
    sel = [np.nonzero(onehot[:, e])[0] for e in range(E)]

    Cmax = max(len(s) for s in sel)
    C = max(((Cmax + P - 1) // P) * P, 2 * P)
    nc = _get_nc(C)

    in_maps = []
    for e in range(E):
        se = sel[e]
        n = len(se)
        xT_e = np.zeros((H, C), np.float32)
        xT_e[:, :n] = xfT[:, se]
        sc_e = np.zeros((C,), np.float32)
        sc_e[:n] = probs[se, e]
        in_maps.append({
            "xT": xT_e,
            "w1": np.ascontiguousarray(w1[e], dtype=np.float32),
            "b1": np.ascontiguousarray(
                np.asarray(b1[e], dtype=np.float32).reshape(FT, P).T),
            "w2": np.ascontiguousarray(w2[e], dtype=np.float32),
            "sc": np.ascontiguousarray(sc_e.reshape(C // P, P).T),
        })

    if PROFILE:
        try:
            r = run_bass_kernel_spmd(nc, in_maps, list(range(E)), trace=True)
        except Exception:
            r = run_bass_kernel_spmd(nc, in_maps, list(range(E)))
    else:
        r = run_bass_kernel_spmd(nc, in_maps, list(range(E)))
    LAST_RESULT = r

    y = np.zeros((T, H), np.float32)
    for e in range(E):
        se = sel[e]
        y[se] += r.results[e]["y"][:len(se)]
    if np.any(b2):
        W = np.zeros((T, E), np.float32)
        W[np.arange(T)[:, None], top2] = probs[np.arange(T)[:, None], top2]
        y += W @ np.asarray(b2, dtype=np.float32)
    return y.reshape(S, B, H)


# revision 12
# speedup vs baseline: 17477.0089x; 17477.0089x over previous
"""MoE feed-forward (dense-routing reference) on 8 trn2 NeuronCores.

Strategy: expert-parallel, one expert per core, exploiting top-2 sparsity —
only tokens routed to an expert are sent to its core (~T*K/E + padding
instead of T tokens per expert, a 4x FLOP cut vs dense all-expert compute).

Host side (numpy): gating (fp64 logits -> softmax -> top-2; the fp64
ordering agrees with any fp32 backend's ordering unless a token's 2nd/3rd
logit gap is ~1e-6 — verified safe for this input distribution), gather of
each expert's tokens into a transposed bf16 [H, C] batch, and scatter-add
of the per-expert outputs. Weights are pre-cast to bf16 on host so the
device DMAs them straight into resident SBUF tiles (no staging/casts).

Device side (Bass/Tile, per core), all matmuls bf16 with fp32 PSUM:
  hiddenT[f, tok] = relu(w1[h, f].T @ xT[h, tok] + b1[f])
  y[tok, h]      = (hiddenT[f, tok].T @ w2[f, h]) * gate[tok]
Weights stay resident in SBUF; tokens stream in chunks of <=512.
"""

import ml_dtypes
import numpy as np

import concourse.bass as bass
import concourse.tile as tile
from concourse import bacc, mybir
from concourse.bass_utils import run_bass_kernel_spmd

BF16 = ml_dtypes.bfloat16
P = 128
H = 1024
F = 4096
E = 8
TOPK = 2
HT = H // P   # 8 contraction tiles for MM1
FT = F // P   # 32 contraction tiles for MM2
TOK = 512     # token chunk (moving-dim N for MM1)
NH = 512      # output column tile for MM2

PROFILE = False          # test harness sets True to try to get a trace
LAST_RESULT = None       # BassKernelResults of last run

_nc_cache = {}


def _chunks(C):
    # Token chunks of 512, avoiding a short 128 tail (moving dims >= 256
    # keep the PE streaming efficient): a remainder of 640 splits 384+256.
    out = []
    c0 = 0
    while c0 < C:
        rem = C - c0
        t = min(TOK, rem) if rem != 640 else 384
        out.append((c0, t))
        c0 += t
    return out


def _build(C):
    """Build + compile the per-core SPMD program for token capacity C."""
    assert C % P == 0
    f32 = mybir.dt.float32
    bf16 = mybir.dt.bfloat16

    nc = bacc.Bacc("TRN2", target_bir_lowering=False, debug=False)
    xT_d = nc.dram_tensor("xT", [H, C], bf16, kind="ExternalInput")
    w1_d = nc.dram_tensor("w1", [H, F], bf16, kind="ExternalInput")
    b1_d = nc.dram_tensor("b1", [P, FT], f32, kind="ExternalInput")
    w2_d = nc.dram_tensor("w2", [F, H], bf16, kind="ExternalInput")
    sc_d = nc.dram_tensor("sc", [P, C // P], f32, kind="ExternalInput")
    y_d = nc.dram_tensor("y", [C, H], f32, kind="ExternalOutput")

    with tile.TileContext(nc) as tc:
        with (
            tc.tile_pool(name="wres", bufs=1) as wres,
            tc.tile_pool(name="consts", bufs=1) as consts,
            tc.tile_pool(name="xpool", bufs=2) as xpool,
            tc.tile_pool(name="hidp", bufs=1) as hidp,
            tc.tile_pool(name="outp", bufs=3) as outp,
            tc.tile_pool(name="psum1", bufs=4, space="PSUM") as psum1,
            tc.tile_pool(name="psum2", bufs=4, space="PSUM") as psum2,
        ):
            w1_sb = wres.tile([P, HT, F], bf16)      # 64KB/partition
            w2_sb = wres.tile([P, FT, H], bf16)      # 64KB/partition
            b1_sb = consts.tile([P, FT], f32)
            sc_sb = consts.tile([P, C // P], f32)

            nc.gpsimd.dma_start(out=b1_sb, in_=b1_d[:, :])
            nc.gpsimd.dma_start(out=sc_sb, in_=sc_d[:, :])

            # Resident weights, straight from DRAM (already bf16).
            # w1 lands f-major (1024-wide column pieces) so MM1 of the first
            # chunk can start as soon as the first pieces arrive; pieces
            # alternate between the two HWDGE rings for landing rate. w2
            # isn't needed until chunk-0 MM2, so it's emitted (= prioritized)
            # after chunk-0's MM1 work, below.
            for q in range(4):
                for h in range(HT):
                    nc.sync.dma_start(
                        out=w1_sb[:, h, q * 1024:(q + 1) * 1024],
                        in_=w1_d[h * P:(h + 1) * P, q * 1024:(q + 1) * 1024],
                    )

            first_chunk = True
            x_engs = [nc.gpsimd, nc.scalar]
            for ci, (c0, t) in enumerate(_chunks(C)):
                xb = xpool.tile([P, HT, TOK], bf16, tag="xb")
                hid_sb = hidp.tile([P, FT, TOK], bf16, tag="hid")
                for h in range(HT):
                    x_engs[ci % 2].dma_start(
                        out=xb[:, h, :t], in_=xT_d[h * P:(h + 1) * P, c0:c0 + t]
                    )

                # MM1: hiddenT[f, tok] = relu(w1.T @ xT + b1)
                for i in range(FT):
                    ps = psum1.tile([P, TOK], f32, tag="ps1")
                    for h in range(HT):
                        nc.tensor.matmul(
                            ps[:, :t],
                            w1_sb[:, h, i * P:(i + 1) * P],
                            xb[:, h, :t],
                            start=(h == 0),
                            stop=(h == HT - 1),
                        )
                    nc.scalar.activation(
                        hid_sb[:, i, :t],
                        ps[:, :t],
                        mybir.ActivationFunctionType.Relu,
                        bias=b1_sb[:, i:i + 1],
                    )

                if first_chunk:
                    first_chunk = False
                    for i in range(FT):
                        nc.gpsimd.dma_start(
                            out=w2_sb[:, i, :], in_=w2_d[i * P:(i + 1) * P, :]
                        )

                # MM2: y[tok, h] = (hiddenT.T @ w2) * gate
                for m in range(t // P):
                    mg = c0 // P + m
                    for nh in range(H // NH):
                        ps = psum2.tile([P, NH], f32, tag="ps2")
                        for i in range(FT):
                            nc.tensor.matmul(
                                ps,
                                hid_sb[:, i, m * P:(m + 1) * P],
                                w2_sb[:, i, nh * NH:(nh + 1) * NH],
                                start=(i == 0),
                                stop=(i == FT - 1),
                            )
                        ot = outp.tile([P, NH], f32, tag="out")
                        nc.vector.tensor_scalar_mul(ot, ps, sc_sb[:, mg:mg + 1])
                        nc.sync.dma_start(
                            out=y_d[c0 + m * P:c0 + (m + 1) * P,
                                    nh * NH:(nh + 1) * NH],
                            in_=ot,
                        )

    nc.compile()
    return nc


def _get_nc(C):
    if C not in _nc_cache:
        _nc_cache[C] = _build(C)
    return _nc_cache[C]


def kernel(x, gate_w, w1, b1, w2, b2):
    global LAST_RESULT
    S, B, _ = x.shape
    T = S * B
    xf = np.ascontiguousarray(x.reshape(T, H), dtype=np.float32)

    # --- host gating (fp64 for backend-stable ordering) ---
    l64 = xf.astype(np.float64) @ gate_w.T.astype(np.float64)
    l64 -= l64.max(axis=-1, keepdims=True)
    p64 = np.exp(l64)
    p64 /= p64.sum(axis=-1, keepdims=True)
    top2 = np.argsort(-l64, axis=-1)[:, :TOPK]
    probs = p64.astype(np.float32)

    xfT = np.ascontiguousarray(xf.T.astype(BF16))  # [H, T] bf16
    onehot = np.zeros((T, E), dtype=bool)
    onehot[np.arange(T)[:, None], top2] = True
    sel = [np.nonzero(onehot[:, e])[0] for e in range(E)]

    Cmax = max(len(s) for s in sel)
    C = max(((Cmax + P - 1) // P) * P, 2 * P)
    nc = _get_nc(C)

    in_maps = []
    for e in range(E):
        se = sel[e]
        n = len(se)
        xT_e = np.zeros((H, C), BF16)
        xT_e[:, :n] = xfT[:, se]
        sc_e = np.zeros((C,), np.float32)
        sc_e[:n] = probs[se, e]
        in_maps.append({
            "xT": xT_e,
            "w1": np.asarray(w1[e], dtype=np.float32).astype(BF16),
            "b1": np.ascontiguousarray(
                np.asarray(b1[e], dtype=np.float32).reshape(FT, P).T),
            "w2": np.asarray(w2[e], dtype=np.float32).astype(BF16),
            "sc": np.ascontiguousarray(sc_e.reshape(C // P, P).T),
        })

    if PROFILE:
        try:
            r = run_bass_kernel_spmd(nc, in_maps, list(range(E)), trace=True)
        except Exception:
            r = run_bass_kernel_spmd(nc, in_maps, list(range(E)))
    else:
        r = run_bass_kernel_spmd(nc, in_maps, list(range(E)))
    LAST_RESULT = r

    y = np.zeros((T, H), np.float32)
    for e in range(E):
        se = sel[e]
        y[se] += r.results[e]["y"][:len(se)]
    if np.any(b2):
        W = np.zeros((T, E), np.float32)
        W[np.arange(T)[:, None], top2] = probs[np.arange(T)[:, None], top2]
        y += W @ np.asarray(b2, dtype=np.float32)
    return y.reshape(S, B, H)
